# revision 46
# baseline (speedup 1.0000x reference)
# Self-contained 8-core Trainium2 Bass kernel for the 2-layer GAT + mean-pool
# problem (nn_GAT_83820581749190).
#
# Sharding: destination nodes (and all their incident edges) are partitioned
# across the 8 cores, so each layer's attention softmax and aggregation
# complete locally per core. Each core builds a replicated layer-1 feature
# table h1 (bf16, 256-byte logical rows) in HBM with a replicated x @ W1
# matmul, then edge-gathers PAIRS of rows (512B per descriptor, index =
# row//2, int16-safe) with the GPSIMD dma_gather custom op; host-precomputed
# parity masks select the correct half downstream. Attention logits are
# computed on-chip (DVE dot with a_src/a_dst), the edge softmax runs without
# segment-max (logits are small; pad slots use a patch row whose h gives
# al_src=-100), and aggregation is identity-matmul PSUM accumulation
# (destinations on partitions via degree-bucketed groups of 128).
# Layer-2 features are exchanged with an AllGather; the same paired-row
# gather runs against the fp32 layer-2 table; mean-pool is a matmul against
# a host-built one-hot graph matrix plus a tiny AllReduce.
import numpy as np
import ml_dtypes

N = 50000
E = 800000
IN = 128
HID = 32
HEADS = 4
OUT = 10
GPOOL = 64
NEG = 0.2
NCORES = 8
S = N // NCORES
SPECIAL1 = N          # layer-1 patch row (h chosen so h . a_src = -100)
SPECIAL2 = 0          # layer-2 patch row (al_src column = -100)
SPECIAL_ALS = -100.0
SB_BLOCK_BUDGET = 24  # max gather blocks per superblock
XCHUNK = 512
PHASES = 99
L1STEP = 99

bf16 = ml_dtypes.bfloat16


def _ceil_to(v, m):
    return (v + m - 1) // m * m


# ======================= host prep =========================================

def _build_layer(src, dstl):
    deg = np.bincount(dstl, minlength=S)
    P = np.argsort(-deg, kind="stable")
    Ppos = np.empty(S, np.int64)
    Ppos[P] = np.arange(S)
    ng = (S + 127) // 128
    D = np.zeros(ng, np.int64)
    dp = deg[P]
    for g in range(ng):
        D[g] = dp[g * 128:(g + 1) * 128].max()
    assert (D > 0).all()
    return dict(src=src, dstl=dstl, deg=deg, P=P, Ppos=Ppos, D=D)


def _emit_slots(l, Dg, row_of_src, special_row):
    """Per group g: rows[g] [D[g],128] of table ROW ids (special_row pads),
    plus slot2cmp mapping output slots -> compacted dst ids."""
    NG = len(Dg)
    Ppos = l["Ppos"]
    nreal = S
    slot2cmp = np.full(NG * 128, -1, np.int64)
    slot2cmp[:nreal] = np.arange(nreal)
    rows = [np.full((int(Dg[g]), 128), special_row, np.int64)
            for g in range(NG)]
    slot_of_edge = Ppos[l["dstl"]]
    order = np.argsort(slot_of_edge, kind="stable")
    so = slot_of_edge[order]
    sr = row_of_src[l["src"][order]]
    jj = np.arange(len(so)) - np.searchsorted(so, so, side="left")
    gg, kk = so // 128, so % 128
    for g in range(NG):
        sel = gg == g
        if sel.any():
            rows[g][jj[sel], kk[sel]] = sr[sel]
    return rows, slot2cmp


def _emit_slots_m(l, Dg, row_of, mate, special_row):
    """Slot emit with co-pair merging: consecutive same-dst edges whose
    sources are mates share one slot (both halves used)."""
    NG = len(Dg)
    Ppos = l["Ppos"]
    slot2cmp = np.full(NG * 128, -1, np.int64)
    slot2cmp[:S] = np.arange(S)
    rows = [np.full((int(Dg[g]), 128), special_row, np.int64)
            for g in range(NG)]
    ue = [np.ones((int(Dg[g]), 128), np.float32) for g in range(NG)]
    uo = [np.zeros((int(Dg[g]), 128), np.float32) for g in range(NG)]
    order = np.argsort(l["dstl"] * (N + 1) + l["src"], kind="stable")
    ds = l["dstl"][order]
    ss = l["src"][order]
    n = len(ds)
    fill = np.zeros(S, np.int64)
    i = 0
    while i < n:
        d = ds[i]
        u = ss[i]
        if i + 1 < n and ds[i + 1] == d and mate[u] == ss[i + 1]:
            r = row_of[u]
            assert r // 2 == row_of[ss[i + 1]] // 2
            rv, e_, o_ = (r // 2) * 2, 1.0, 1.0
            i += 2
        else:
            rv = row_of[u]
            e_, o_ = 1.0 - (rv % 2), float(rv % 2)
            i += 1
        slot = Ppos[d]
        g, k = slot // 128, slot % 128
        j = fill[d]
        fill[d] += 1
        rows[g][j, k] = rv
        ue[g][j, k] = e_
        uo[g][j, k] = o_
    assert (fill == l["deg"]).all()
    return rows, ue, uo, slot2cmp


def _wrap_masks(arrs):
    segs = [np.ascontiguousarray(a.T) for a in arrs if a.size]
    return (np.concatenate(segs, axis=1).astype(np.float32) if segs
            else np.zeros((128, 0), np.float32))


def _wrap16(idx):
    """[n] -> [128, n//16] int16: idx i at [i%16, i//16], replicated x8."""
    n = len(idx)
    assert n % 16 == 0
    w = np.ascontiguousarray(np.asarray(idx).reshape(n // 16, 16).T)
    w = w.astype(np.int16)
    return np.tile(w, (8, 1))


def _wrap_rows(rows_arrs):
    """idx stream (row//2) wrapped, plus even-parity masks [128, NB]."""
    idx_segs = []
    pme_segs = []
    for a in rows_arrs:
        if a.size:
            assert (a // 2 <= 32767).all()
            idx_segs.append(_wrap16((a // 2).reshape(-1)))
            pme_segs.append(np.ascontiguousarray((1 - (a % 2)).T))
    w_idx = (np.concatenate(idx_segs, axis=1) if idx_segs
             else np.zeros((128, 0), np.int16))
    pme = (np.concatenate(pme_segs, axis=1).astype(np.float32) if pme_segs
           else np.zeros((128, 0), np.float32))
    return w_idx, pme


def host_prep(x, edge_index, batch, W1, a1_src, a1_dst, b1, W2, a2_src, a2_dst,
              b2, Wl, bl):
    x = np.asarray(x, np.float32)
    edge_index = np.asarray(edge_index, np.int64)
    batch = np.asarray(batch, np.int64)
    src_all = np.concatenate([edge_index[0], np.arange(N, dtype=np.int64)])
    dst_all = np.concatenate([edge_index[1], np.arange(N, dtype=np.int64)])
    owner = dst_all // S

    a1_src = np.asarray(a1_src, np.float32)
    a1_dst = np.asarray(a1_dst, np.float32)
    W1 = np.asarray(W1, np.float32)
    W2 = np.asarray(W2, np.float32)
    W2ext = np.concatenate(
        [W2, W2 @ np.asarray(a2_src, np.float32)[0][:, None],
         W2 @ np.asarray(a2_dst, np.float32)[0][:, None]], axis=1)  # [128,34]

    # a1x: [0:128]=a_src flat, [128:256]=a_src flat, [256:384]=a_dst flat
    asf = a1_src.reshape(-1)
    adf = a1_dst.reshape(-1)
    a1x = np.tile(np.concatenate([asf, asf, adf])[None, :], (128, 1))

    # layer-1 patch row: h with h . a_src[h] = -100 for every head
    hp = np.concatenate([SPECIAL_ALS * a1_src[h] / (a1_src[h] ** 2).sum()
                         for h in range(HEADS)])
    assert np.abs(hp).max() < 1e4
    patch1 = np.tile(hp[None, :], (1, 1))

    cores = [dict(c=c) for c in range(NCORES)]
    for cd in cores:
        c = cd["c"]
        m = owner == c
        cd["src"] = src_all[m]
        cd["dstl"] = dst_all[m] - c * S

    # ---------- layer 1 ----------
    # Co-pair foreign sources that feed the same destination so both edges
    # share one 512B pair-descriptor (slot uses BOTH halves).
    for cd in cores:
        c = cd["c"]
        src, dstl = cd["src"], cd["dstl"]
        foreign = (src // S) != c
        order = np.argsort(dstl * (N + 1) + src, kind="stable")
        ds, ss, fs = dstl[order], src[order], foreign[order]
        mate = np.full(N, -1, np.int64)
        same = (ds[:-1] == ds[1:]) & fs[:-1] & fs[1:] & (ss[:-1] != ss[1:])
        cand_u = ss[:-1][same]
        cand_v = ss[1:][same]
        for u, v in zip(cand_u, cand_v):
            if mate[u] < 0 and mate[v] < 0:
                mate[u] = v
                mate[v] = u
        deg = np.bincount(dstl, minlength=S)
        merged = mate[cand_u] == cand_v
        dmerge = np.bincount(ds[:-1][same][merged], minlength=S)
        degp = deg - dmerge
        P = np.argsort(-degp, kind="stable")
        Ppos = np.empty(S, np.int64)
        Ppos[P] = np.arange(S)
        ng = (S + 127) // 128
        D = np.zeros(ng, np.int64)
        dp = degp[P]
        for g in range(ng):
            D[g] = dp[g * 128:(g + 1) * 128].max()
        cd["l1"] = dict(src=src, dstl=dstl, deg=degp, P=P, Ppos=Ppos, D=D)
        cd["mate1"] = mate
        pos_of = np.empty(N, np.int64)
        own_mask = np.zeros(N, bool)
        own_mask[c * S:(c + 1) * S] = True
        pos_of[c * S:(c + 1) * S] = Ppos
        fnodes = np.where(~own_mask)[0]
        is_m = mate[fnodes] >= 0
        mlist = fnodes[is_m]
        lo = mlist[mlist < mate[mlist]]
        nxt = S
        for u in lo:
            pos_of[u] = nxt
            pos_of[mate[u]] = nxt + 1
            nxt += 2
        singles = fnodes[~is_m]
        pos_of[singles] = nxt + np.arange(len(singles))
        cd["row_of"] = pos_of
    NG1 = max(len(cd["l1"]["D"]) for cd in cores)
    D1 = np.zeros(NG1, np.int64)
    for cd in cores:
        d = cd["l1"]["D"]
        D1[:len(d)] = np.maximum(D1[:len(d)], d)
    for cd in cores:
        cd["rows1"], cd["ue1"], cd["uo1"], cd["slot2cmp1"] = _emit_slots_m(
            cd["l1"], D1, cd["row_of"], cd["mate1"], SPECIAL1)

    # ---------- layer 2 ----------
    # layer-2 features live in a blocked bf16 table: core c's partition p,
    # group g at flat row (c*128+p)*NG2 + g (64 bf16 each; pairs of flat
    # rows share one 256B gather descriptor)
    for cd in cores:
        cd["l2"] = _build_layer(cd["src"], cd["dstl"])
    NG2 = max(len(cd["l2"]["D"]) for cd in cores)
    D2 = np.zeros(NG2, np.int64)
    for cd in cores:
        d = cd["l2"]["D"]
        D2[:len(d)] = np.maximum(D2[:len(d)], d)
    flat2_of = np.empty(N, np.int64)
    for cd in cores:
        c = cd["c"]
        q = cd["l2"]["Ppos"]
        flat2_of[c * S:(c + 1) * S] = \
            (c * 128 + q % 128) * NG2 + q // 128
    for cd in cores:
        c = cd["c"]
        special2 = (c * 128 + S % 128) * NG2 + S // 128  # own trash row
        cd["rows2"], cd["slot2cmp2"] = _emit_slots(
            cd["l2"], D2, flat2_of, special2)

    # ---------- aux ----------
    cnt = np.bincount(batch, minlength=GPOOL).astype(np.float32)
    recip_cnt = (1.0 / np.maximum(cnt, 1.0)).astype(np.float32)

    XT_COLS = _ceil_to(N + 2, XCHUNK)
    for cd in cores:
        c = cd["c"]
        gids = batch[c * S:(c + 1) * S]
        Mp = np.zeros((NG2 * 128, GPOOL), np.float32)
        s2c = cd["slot2cmp2"]
        real = s2c >= 0
        Mp[np.where(real)[0], gids[cd["l2"]["P"][s2c[real]]]] = 1.0
        cd["mpool"] = Mp.astype(bf16)

        s2c1 = cd["slot2cmp1"]
        tgt = np.full(len(s2c1), S, np.int64)  # trash row for dummy slots
        r1 = s2c1 >= 0
        tgt[r1] = cd["l2"]["Ppos"][cd["l1"]["P"][s2c1[r1]]]

        xt = np.zeros((IN, XT_COLS), np.float32)
        xt[:, cd["row_of"]] = x.T
        cd["xT"] = xt.astype(bf16)

        idx1_segs = [_wrap16((a // 2).reshape(-1)) for a in cd["rows1"]
                     if a.size]
        cd["w_idx1"] = (np.concatenate(idx1_segs, axis=1) if idx1_segs
                        else np.zeros((128, 0), np.int16))
        cd["ue1w"] = _wrap_masks(cd["ue1"])
        cd["uo1w"] = _wrap_masks(cd["uo1"])
        cd["w_idx2"], cd["pme2"] = _wrap_rows(cd["rows2"])
        cd["w_scat1"] = _wrap16(tgt)

    # written over the trash row after the scatter: al_src=-100 kills pads
    patch2 = np.zeros((1, 64), np.float32)
    patch2[0, 32] = SPECIAL_ALS

    return dict(cores=cores,
                D1=[int(v) for v in D1], D2=[int(v) for v in D2],
                W1=W1.astype(bf16), W2ext=W2ext.astype(bf16),
                Wl=np.asarray(Wl, np.float32),
                a1x=a1x.astype(bf16),
                b1=np.tile(np.asarray(b1, np.float32).reshape(1, -1),
                           (128, 1)),
                b2=np.tile(np.asarray(b2, np.float32).reshape(1, -1),
                           (128, 1)),
                bl=np.tile(np.asarray(bl, np.float32).reshape(1, -1),
                           (GPOOL, 1)),
                rcnt=np.tile(recip_cnt.reshape(1, -1), (HID, 1)),
                patch1=patch1.astype(bf16), patch2=patch2,
                ident=np.eye(128, dtype=bf16))


def _pack_superblocks(D, budget=SB_BLOCK_BUDGET):
    sbs, cur, tot = [], [], 0
    for g in range(len(D)):
        d = int(D[g])
        if cur and tot + d > budget:
            sbs.append(cur)
            cur, tot = [], 0
        cur.append(g)
        tot += d
    if cur:
        sbs.append(cur)
    return sbs


def make_sched(prep):
    D1, D2 = prep["D1"], prep["D2"]
    return dict(D1=D1, D2=D2,
                SB1=_pack_superblocks(D1), SB2=_pack_superblocks(D2),
                HASB1=bool(np.any(prep["b1"])), HASB2=bool(np.any(prep["b2"])),
                HASBL=bool(np.any(prep["bl"])))


# ======================= bass kernel =======================================

def build_bass(sc):
    import concourse.bacc as bacc
    import concourse.tile as tile
    import concourse.mybir as mybir
    from concourse.library_config import mlp

    dt = mybir.dt
    Alu = mybir.AluOpType
    Act = mybir.ActivationFunctionType
    Axis = mybir.AxisListType

    D1, D2 = sc["D1"], sc["D2"]
    SB1, SB2 = sc["SB1"], sc["SB2"]
    HASB1 = sc.get("HASB1", True)
    HASB2 = sc.get("HASB2", True)
    HASBL = sc.get("HASBL", True)
    NG1, NG2 = len(D1), len(D2)
    XT_COLS = _ceil_to(N + 2, XCHUNK)
    NCHUNK = XT_COLS // XCHUNK
    SH2_ROWS = _ceil_to(S + 2, 128)
    NB1 = sum(D1)
    NB2 = sum(D2)
    o1 = np.concatenate([[0], np.cumsum(D1)]).astype(int)
    o2 = np.concatenate([[0], np.cumsum(D2)]).astype(int)

    nc = bacc.Bacc("TRN2", target_bir_lowering=False, debug=False,
                   num_devices=NCORES, num_swdge_queues=4)

    t_xT = nc.dram_tensor("xT", [IN, XT_COLS], dt.bfloat16,
                          kind="ExternalInput")
    t_w1 = nc.dram_tensor("w1", [IN, IN], dt.bfloat16, kind="ExternalInput")
    t_w2 = nc.dram_tensor("w2ext", [IN, 34], dt.bfloat16,
                          kind="ExternalInput")
    t_wl = nc.dram_tensor("wl", [HID, OUT], dt.float32, kind="ExternalInput")
    t_a1x = nc.dram_tensor("a1x", [128, 384], dt.bfloat16,
                           kind="ExternalInput")
    t_b1 = nc.dram_tensor("b1", [128, HEADS * HID], dt.float32,
                          kind="ExternalInput")
    t_b2 = nc.dram_tensor("b2", [128, HID], dt.float32, kind="ExternalInput")
    t_bl = nc.dram_tensor("bl", [GPOOL, OUT], dt.float32,
                          kind="ExternalInput")
    t_rcnt = nc.dram_tensor("rcnt", [HID, GPOOL], dt.float32,
                            kind="ExternalInput")
    t_patch1 = nc.dram_tensor("patch1", [1, 128], dt.bfloat16,
                              kind="ExternalInput")
    t_patch2 = nc.dram_tensor("patch2", [1, 64], dt.float32,
                              kind="ExternalInput")
    t_ident = nc.dram_tensor("ident", [128, 128], dt.bfloat16,
                             kind="ExternalInput")
    t_mpool = nc.dram_tensor("mpool", [NG2 * 128, GPOOL], dt.bfloat16,
                             kind="ExternalInput")
    n1 = max(8 * NB1, 8)
    n2 = max(8 * NB2, 8)
    t_i1 = nc.dram_tensor("idx1", [128, n1], dt.int16, kind="ExternalInput")
    t_i2 = nc.dram_tensor("idx2", [128, n2], dt.int16, kind="ExternalInput")
    t_ue1 = nc.dram_tensor("ue1", [128, max(NB1, 1)], dt.bfloat16,
                           kind="ExternalInput")
    t_uo1 = nc.dram_tensor("uo1", [128, max(NB1, 1)], dt.bfloat16,
                           kind="ExternalInput")
    t_pm2 = nc.dram_tensor("pme2", [128, max(NB2, 1)], dt.float32,
                           kind="ExternalInput")
    t_scat1 = nc.dram_tensor("scat1", [128, 8 * NG1], dt.int16,
                             kind="ExternalInput")
    t_out = nc.dram_tensor("out", [GPOOL, OUT], dt.float32,
                           kind="ExternalOutput")

    rg = [list(range(NCORES))]
    _qc = [0]

    def nextq():
        _qc[0] = (_qc[0] + 1) % 4
        return _qc[0]

    with tile.TileContext(nc) as tc:
        with (
            tc.tile_pool(name="const", bufs=1) as constp,
            tc.tile_pool(name="pre", bufs=1) as prep_pool,
            tc.tile_pool(name="dram", bufs=1, space="DRAM") as dramp,
        ):
            nc.gpsimd.load_library(mlp)

            # logical row-major tables; gathers view them as paired rows
            table1 = dramp.tile([XT_COLS, 128], dt.bfloat16, tag="table1")
            h2b = dramp.tile([128, NG2 * 64], dt.bfloat16, tag="h2b")
            table2b = dramp.tile([128 * NCORES * NG2 * 64], dt.bfloat16,
                                 tag="table2b")
            h2sh = dramp.tile([SH2_ROWS, 64], dt.float32, tag="h2sh")
            cc_in = dramp.tile([HID, GPOOL], dt.float32, tag="ccin")
            cc_out = dramp.tile([HID, GPOOL], dt.float32, tag="ccout")

            w1_t = constp.tile([IN, IN], dt.bfloat16)
            nc.sync.dma_start(w1_t[:], t_w1[:])
            w2_t = constp.tile([IN, 34], dt.bfloat16)
            nc.sync.dma_start(w2_t[:], t_w2[:])
            wl_t = constp.tile([HID, OUT], dt.float32)
            nc.sync.dma_start(wl_t[:], t_wl[:])
            a1x_t = constp.tile([128, 384], dt.bfloat16)
            nc.sync.dma_start(a1x_t[:], t_a1x[:])
            b1_t = constp.tile([128, HEADS * HID], dt.float32)
            nc.sync.dma_start(b1_t[:], t_b1[:])
            b2_t = constp.tile([128, HID], dt.float32)
            nc.sync.dma_start(b2_t[:], t_b2[:])
            bl_t = constp.tile([GPOOL, OUT], dt.float32)
            nc.sync.dma_start(bl_t[:], t_bl[:])
            rc_t = constp.tile([HID, GPOOL], dt.float32)
            nc.sync.dma_start(rc_t[:], t_rcnt[:])
            id_t = constp.tile([128, 128], dt.bfloat16)
            nc.sync.dma_start(id_t[:], t_ident[:])

            # preload all gather indices and parity masks
            i1_all = prep_pool.tile([128, n1], dt.int16)
            nc.sync.dma_start(i1_all[:], t_i1[:])
            ue1_t = prep_pool.tile([128, max(NB1, 1)], dt.bfloat16)
            nc.sync.dma_start(ue1_t[:], t_ue1[:])
            uo1_t = prep_pool.tile([128, max(NB1, 1)], dt.bfloat16)
            nc.sync.dma_start(uo1_t[:], t_uo1[:])
            i2_all = prep_pool.tile([128, n2], dt.int16)
            pm2_t = prep_pool.tile([128, max(NB2, 1)], dt.float32)
            scat1_t = prep_pool.tile([128, 8 * NG1], dt.int16)
            nc.sync.dma_start(scat1_t[:], t_scat1[:])
            ald1_t = prep_pool.tile([128, NG1, 4], dt.float32)
            ald2_t = prep_pool.tile([128, NG2, 1], dt.float32)

            # zero the scatter_add target
            with tc.tile_pool(name="zp", bufs=1) as zp:
                z_t = zp.tile([128, SH2_ROWS // 128 * 64], dt.float32)
                nc.vector.memset(z_t[:], 0.0)
                nc.sync.dma_start(
                    h2sh[:, :].rearrange("(p k) e -> p (k e)", p=128), z_t[:])

            # ---------------- phase X: build table1 ----------------
            with (
                tc.tile_pool(name="xload", bufs=3) as xlp,
                tc.tile_pool(name="xout", bufs=3) as xop,
                tc.tile_pool(name="xpsum", bufs=4, space="PSUM") as xpp,
            ):
                for t in range(NCHUNK):
                    # alternate loads/writes across the two HWDGE rings so
                    # neither ring serializes the whole 25.6MB stream
                    ld_eng = nc.sync if t % 2 == 0 else nc.scalar
                    st_eng = nc.scalar if t % 2 == 0 else nc.sync
                    xt_t = xlp.tile([IN, XCHUNK], dt.bfloat16, tag="xt")
                    ld_eng.dma_start(xt_t[:],
                                     t_xT[:, t * XCHUNK:(t + 1) * XCHUNK])
                    o_t = xop.tile([128, 4, 128], dt.bfloat16, tag="xo")
                    for k in range(4):
                        ps = xpp.tile([128, 128], dt.float32, tag="xp")
                        nc.tensor.matmul(ps[:], xt_t[:, k * 128:(k + 1) * 128],
                                         w1_t[:], start=True, stop=True)
                        if k % 2 == 0:
                            nc.vector.tensor_copy(o_t[:, k, :], ps[:])
                        else:
                            nc.scalar.activation(o_t[:, k, :], ps[:],
                                                 Act.Copy)
                        gix = 4 * t + k
                        if gix < NG1:
                            # own-row ald = h . a_dst, straight off the PSUM
                            ap_t = xop.tile([128, 128], dt.bfloat16,
                                            tag="apr")
                            nc.vector.tensor_tensor(
                                ap_t[:], ps[:], a1x_t[:, 256:384], Alu.mult)
                            nc.vector.tensor_reduce(
                                ald1_t[:, gix, :],
                                ap_t[:].rearrange("p (h c) -> p h c", h=4),
                                axis=Axis.X, op=Alu.add)
                    st_eng.dma_start(
                        table1[t * XCHUNK:(t + 1) * XCHUNK, :].rearrange(
                            "(k p) e -> p k e", p=128), o_t[:])
            with tc.tile_pool(name="patchp", bufs=1) as pp:
                p1_t = pp.tile([1, 128], dt.bfloat16)
                nc.sync.dma_start(p1_t[:], t_patch1[:])
                nc.sync.dma_start(table1[SPECIAL1:SPECIAL1 + 1, :],
                                  p1_t[0:1, :])
            p2_t = prep_pool.tile([1, 64], dt.float32)
            nc.sync.dma_start(p2_t[:], t_patch2[:])

            if PHASES >= 2:
                # ---------------- phase L1: edges ----------------
                tab1p = table1[:, :].rearrange("(a h) c -> a (h c)", h=2)
                Dmax1 = max(D1)
                NBSB1 = max(sum(D1[g] for g in sb) for sb in SB1)
                with (
                    tc.tile_pool(name="gath1", bufs=6) as gathp,
                    tc.tile_pool(name="als1", bufs=2) as alsp,
                    tc.tile_pool(name="small1", bufs=3) as smallp,
                    tc.tile_pool(name="epi1", bufs=3) as epip,
                    tc.tile_pool(name="scatp", bufs=1) as scatp,
                    tc.tile_pool(name="agg1", bufs=2, space="PSUM") as aggp,
                    tc.tile_pool(name="psT1", bufs=2, space="PSUM") as psTp,
                    tc.tile_pool(name="ps21", bufs=2, space="PSUM") as ps2p,
                ):
                    scat_t = scatp.tile([128, NG1, 64], dt.float32, tag="sc")
                    nc.vector.memset(scat_t[:], 0.0)
                    elu_all = scatp.tile([128, NG1, 128], dt.bfloat16,
                                         tag="eluall")
                    scat_done = [0]

                    def flush_scatter(upto):
                        g0s = scat_done[0]
                        ngk = upto - g0s
                        if ngk <= 0:
                            return
                        nc.gpsimd.dma_scatter_add(
                            h2sh[0:S + 1, :], scat_t[:, g0s:upto, :],
                            scat1_t[:, 8 * g0s:8 * upto],
                            128 * ngk, 128 * ngk, 64,
                            single_packet=False, queue_num=nextq())
                        scat_done[0] = upto

                    for sb in SB1:
                        g0 = sb[0]
                        nb = sum(D1[g] for g in sb)
                        boff = o1[g0]
                        gb_t = gathp.tile([128, NBSB1, 256],
                                          dt.bfloat16, tag="gb")
                        nc.gpsimd.dma_gather(
                            gb_t[:, :nb, :], tab1p,
                            i1_all[:, 8 * boff:8 * (boff + nb)],
                            128 * nb, 128 * nb, 256,
                            single_packet=False, queue_num=nextq())
                        if L1STEP < 2:
                            continue
                        # al_src for both pair-halves: prod + reduce
                        prod_t = alsp.tile([128, NBSB1, 256], dt.bfloat16,
                                           tag="prod")
                        als8_t = alsp.tile([128, NBSB1, 8], dt.float32,
                                           tag="als8")
                        nc.vector.tensor_tensor(
                            prod_t[:, :nb, :], gb_t[:, :nb, :],
                            a1x_t[:, 0:256].unsqueeze(1).broadcast_to(
                                (128, nb, 256)), Alu.mult)
                        nc.vector.tensor_reduce(
                            als8_t[:, :nb, :],
                            prod_t[:, :nb, :].rearrange(
                                "p b (j c) -> p b j c", j=8),
                            axis=Axis.X, op=Alu.add)
                        off = 0
                        for gi, g in enumerate(sb):
                            D = D1[g]
                            if L1STEP < 3:
                                off += D
                                continue
                            logit_t = smallp.tile([128, Dmax1, 8], dt.float32,
                                                  tag="lg")
                            exb_t = smallp.tile([128, Dmax1, 8], dt.bfloat16,
                                                tag="exb")
                            exe_t = smallp.tile([128, Dmax1, 4], dt.bfloat16,
                                                tag="exe")
                            exo_t = smallp.tile([128, Dmax1, 4], dt.bfloat16,
                                                tag="exo")
                            sum_t = smallp.tile([128, Dmax1, 4], dt.bfloat16,
                                                tag="sm")
                            den_t = smallp.tile([128, 4], dt.float32,
                                                tag="dn")
                            rec_t = smallp.tile([128, 4], dt.float32,
                                                tag="rc")
                            ald_ap = ald1_t[:, g, :]
                            ald_b = ald_ap.unsqueeze(1).broadcast_to(
                                (128, D, 4))
                            # independent logits for both pair halves
                            nc.vector.scalar_tensor_tensor(
                                logit_t[:, :D, 0:4],
                                als8_t[:, off:off + D, 0:4], 0.0,
                                ald_b, Alu.add, Alu.add)
                            nc.vector.scalar_tensor_tensor(
                                logit_t[:, :D, 4:8],
                                als8_t[:, off:off + D, 4:8], 0.0,
                                ald_b, Alu.add, Alu.add)
                            nc.vector.scalar_tensor_tensor(
                                logit_t[:, :D, :], logit_t[:, :D, :], NEG,
                                logit_t[:, :D, :], Alu.mult, Alu.max)
                            nc.scalar.activation(exb_t[:, :D, :],
                                                 logit_t[:, :D, :], Act.Exp)
                            ue_b = ue1_t[:, boff + off:boff + off + D
                                         ].unsqueeze(2).broadcast_to(
                                             (128, D, 4))
                            uo_b = uo1_t[:, boff + off:boff + off + D
                                         ].unsqueeze(2).broadcast_to(
                                             (128, D, 4))
                            nc.vector.tensor_tensor(
                                exe_t[:, :D, :], exb_t[:, :D, 0:4], ue_b,
                                Alu.mult)
                            nc.vector.tensor_tensor(
                                exo_t[:, :D, :], exb_t[:, :D, 4:8], uo_b,
                                Alu.mult)
                            nc.vector.tensor_tensor(
                                sum_t[:, :D, :], exe_t[:, :D, :],
                                exo_t[:, :D, :], Alu.add)
                            nc.vector.tensor_reduce(
                                den_t[:], sum_t[:, :D, :].transpose([0, 2, 1]),
                                axis=Axis.X, op=Alu.add)
                            nc.vector.reciprocal(rec_t[:], den_t[:])
                            if L1STEP < 4:
                                off += D
                                continue
                            h_e = gb_t[:, off:off + D, 0:128].rearrange(
                                "p b (h c) -> p b h c", h=4)
                            nc.vector.tensor_tensor(
                                h_e, h_e,
                                exe_t[:, :D, :].unsqueeze(3).broadcast_to(
                                    (128, D, 4, HID)), Alu.mult)
                            h_o = gb_t[:, off:off + D, 128:256].rearrange(
                                "p b (h c) -> p b h c", h=4)
                            nc.vector.tensor_tensor(
                                h_o, h_o,
                                exo_t[:, :D, :].unsqueeze(3).broadcast_to(
                                    (128, D, 4, HID)), Alu.mult)
                            if L1STEP < 5:
                                off += D
                                continue
                            agg = aggp.tile([128, 128], dt.float32, tag="agg")
                            for bi in range(2 * D):
                                rhs = gb_t[:, off + bi // 2,
                                           (bi % 2) * 128:(bi % 2 + 1) * 128]
                                nc.tensor.matmul(agg[:], id_t[:], rhs,
                                                 start=(bi == 0),
                                                 stop=(bi == 2 * D - 1))
                            scaled_t = epip.tile([128, 128], dt.float32,
                                                 tag="sd")
                            nc.vector.tensor_tensor(
                                scaled_t[:].rearrange("p (h c) -> p h c", h=4),
                                agg[:].rearrange("p (h c) -> p h c", h=4),
                                rec_t[:].unsqueeze(2).broadcast_to(
                                    (128, 4, HID)), Alu.mult)
                            if HASB1:
                                nc.vector.tensor_tensor(
                                    scaled_t[:], scaled_t[:], b1_t[:],
                                    Alu.add)
                            tmp_t = epip.tile([128, 128], dt.float32,
                                              tag="tm")
                            nc.scalar.activation(tmp_t[:], scaled_t[:],
                                                 Act.Relu, scale=-1.0)
                            nc.scalar.activation(tmp_t[:], tmp_t[:], Act.Exp,
                                                 scale=-1.0)
                            nc.vector.scalar_tensor_tensor(
                                elu_all[:, g, :], tmp_t[:], -1.0, scaled_t[:],
                                Alu.add, Alu.max)
                            off += D
                        # ---- pass 2 for this superblock's groups
                        if L1STEP >= 5:
                            for g in sb:
                                psT = psTp.tile([128, 128], dt.bfloat16,
                                                tag="pt")
                                nc.tensor.transpose(psT[:], elu_all[:, g, :],
                                                    id_t[:])
                                eluT_t = epip.tile([128, 128], dt.bfloat16,
                                                   tag="et")
                                nc.scalar.activation(eluT_t[:], psT[:],
                                                     Act.Copy)
                                ps2 = ps2p.tile([128, 34], dt.float32,
                                                tag="p2")
                                nc.tensor.matmul(ps2[:], eluT_t[:], w2_t[:],
                                                 start=True, stop=True)
                                if g % 2 == 0:
                                    nc.scalar.activation(scat_t[:, g, 0:34],
                                                         ps2[:], Act.Copy)
                                else:
                                    nc.vector.tensor_copy(scat_t[:, g, 0:34],
                                                          ps2[:])
                            # overlap the h2 scatter with the remaining
                            # superblocks (~12-group chunks)
                            if sb[-1] + 1 - scat_done[0] >= 12:
                                flush_scatter(sb[-1] + 1)
                    if L1STEP >= 6:
                        flush_scatter(NG1)
                        # pad slots scatter garbage into the trash row;
                        # overwrite with the al_src=-100 pad row before the
                        # L2 extraction reads it
                        nc.sync.dma_start(h2sh[S:S + 1, :], p2_t[0:1, :])
            if PHASES >= 3:
                # ---------------- exchange ----------------
                # pack own rows to blocked bf16 (also yields the ald column),
                # then AllGather the compact table
                with tc.tile_pool(name="aldtmp2", bufs=1) as atp:
                    atmp = atp.tile([128, NG2, 64], dt.float32)
                    nc.sync.dma_start(
                        atmp[:],
                        h2sh[0:128 * NG2, :].rearrange("(b p) e -> p b e",
                                                       p=128))
                    nc.vector.tensor_copy(ald2_t[:], atmp[:, :, 33:34])
                    atmpb = atp.tile([128, NG2, 64], dt.bfloat16)
                    nc.scalar.activation(atmpb[:], atmp[:], Act.Copy)
                    nc.sync.dma_start(
                        h2b[:, :], atmpb[:].rearrange("p b e -> p (b e)"))
                nc.gpsimd.collective_compute(
                    "AllGather", mybir.AluOpType.bypass, replica_groups=rg,
                    ins=[h2b[:, :]],
                    outs=[table2b[:].rearrange("(r x) -> r x",
                                               r=128 * NCORES)])

            if PHASES >= 4:
                # ---------------- phase L2: edges + pool ----------------
                # preloads below overlap the AllGather
                nc.sync.dma_start(i2_all[:], t_i2[:])
                nc.sync.dma_start(pm2_t[:], t_pm2[:])
                tab2p = table2b[:].rearrange("(y c) -> y c", c=128)
                Dmax2 = max(D2)
                NBSB2 = max(sum(D2[g] for g in sb) for sb in SB2)
                with (
                    tc.tile_pool(name="gath2", bufs=6) as gathp,
                    tc.tile_pool(name="small2", bufs=3) as smallp,
                    tc.tile_pool(name="epi2", bufs=3) as epip,
                    tc.tile_pool(name="agg2", bufs=2, space="PSUM") as aggp,
                    tc.tile_pool(name="poolps", bufs=1,
                                 space="PSUM") as poolpp,
                    tc.tile_pool(name="mp2", bufs=3) as mpp,
                ):
                    poolps = poolpp.tile([HID, GPOOL], dt.float32)
                    h2p_all = mpp.tile([128, NG2, HID], dt.bfloat16,
                                       tag="h2pall", bufs=1)
                    for sb in SB2:
                        g0 = sb[0]
                        nb = sum(D2[g] for g in sb)
                        boff = o2[g0]
                        gb_t = gathp.tile([128, NBSB2, 128], dt.bfloat16,
                                          tag="gb")
                        nc.gpsimd.dma_gather(
                            gb_t[:, :nb, :], tab2p,
                            i2_all[:, 8 * boff:8 * (boff + nb)],
                            128 * nb, 128 * nb, 128, single_packet=False,
                            queue_num=nextq())
                        off = 0
                        for gi, g in enumerate(sb):
                            D = D2[g]
                            logit_t = smallp.tile([128, Dmax2, 1], dt.float32,
                                                  tag="lg")
                            t2_t = smallp.tile([128, Dmax2, 1], dt.float32,
                                               tag="t2")
                            exf_t = smallp.tile([128, Dmax2, 1], dt.float32,
                                                tag="exf")
                            exe_t = smallp.tile([128, Dmax2, 1], dt.float32,
                                                tag="exe")
                            exo_t = smallp.tile([128, Dmax2, 1], dt.float32,
                                                tag="exo")
                            den_t = smallp.tile([128, 1], dt.float32,
                                                tag="dn")
                            rec_t = smallp.tile([128, 1], dt.float32,
                                                tag="rc")
                            ald_ap = ald2_t[:, g, :]
                            pmg = pm2_t[:, boff + off:boff + off + D]
                            pmg_b = pmg.unsqueeze(2)
                            # als = odd - (odd-even)*pme
                            nc.vector.tensor_tensor(
                                t2_t[:, :D, :],
                                gb_t[:, off:off + D, 96:97],
                                gb_t[:, off:off + D, 32:33], Alu.subtract)
                            nc.vector.tensor_tensor(
                                t2_t[:, :D, :], t2_t[:, :D, :], pmg_b,
                                Alu.mult)
                            nc.vector.tensor_tensor(
                                logit_t[:, :D, :],
                                gb_t[:, off:off + D, 96:97],
                                t2_t[:, :D, :], Alu.subtract)
                            nc.vector.tensor_scalar(
                                logit_t[:, :D, :], logit_t[:, :D, :],
                                ald_ap, None, Alu.add)
                            nc.vector.scalar_tensor_tensor(
                                logit_t[:, :D, :], logit_t[:, :D, :], NEG,
                                logit_t[:, :D, :], Alu.mult, Alu.max)
                            nc.scalar.activation(exf_t[:, :D, :],
                                                 logit_t[:, :D, :], Act.Exp)
                            nc.vector.tensor_reduce(
                                den_t[:], exf_t[:, :D, :].transpose([0, 2, 1]),
                                axis=Axis.X, op=Alu.add)
                            nc.vector.reciprocal(rec_t[:], den_t[:])
                            nc.vector.tensor_tensor(
                                exe_t[:, :D, :], exf_t[:, :D, :], pmg_b,
                                Alu.mult)
                            nc.vector.tensor_tensor(
                                exo_t[:, :D, :], exf_t[:, :D, :],
                                exe_t[:, :D, :], Alu.subtract)
                            exh_t = smallp.tile([128, Dmax2, 2, HID],
                                                dt.bfloat16, tag="exh")
                            nc.vector.tensor_tensor(
                                exh_t[:, :D, 0, :],
                                gb_t[:, off:off + D, 0:HID],
                                exe_t[:, :D, :].broadcast_to(
                                    (128, D, HID)), Alu.mult)
                            nc.vector.tensor_tensor(
                                exh_t[:, :D, 1, :],
                                gb_t[:, off:off + D, 64:64 + HID],
                                exo_t[:, :D, :].broadcast_to(
                                    (128, D, HID)), Alu.mult)
                            agg = aggp.tile([128, HID], dt.float32, tag="agg")
                            for bi in range(2 * D):
                                nc.tensor.matmul(
                                    agg[:], id_t[:],
                                    exh_t[:, bi // 2, bi % 2, :],
                                    start=(bi == 0),
                                    stop=(bi == 2 * D - 1))
                            scaled_t = epip.tile([128, HID], dt.float32,
                                                 tag="sd")
                            nc.vector.tensor_scalar(scaled_t[:], agg[:],
                                                    rec_t[:], None, Alu.mult)
                            if HASB2:
                                nc.vector.tensor_tensor(
                                    scaled_t[:], scaled_t[:], b2_t[:],
                                    Alu.add)
                            tmp_t = epip.tile([128, HID], dt.float32,
                                              tag="tm")
                            nc.scalar.activation(tmp_t[:], scaled_t[:],
                                                 Act.Relu, scale=-1.0)
                            nc.scalar.activation(tmp_t[:], tmp_t[:], Act.Exp,
                                                 scale=-1.0)
                            nc.vector.scalar_tensor_tensor(
                                h2p_all[:, g, :], tmp_t[:], -1.0, scaled_t[:],
                                Alu.add, Alu.max)
                            mp_t = mpp.tile([128, GPOOL], dt.bfloat16,
                                            tag="mp")
                            nc.sync.dma_start(
                                mp_t[:], t_mpool[g * 128:(g + 1) * 128, :])
                            nc.tensor.matmul(poolps[:], h2p_all[:, g, :],
                                             mp_t[:], start=(g == 0),
                                             stop=(g == NG2 - 1))
                            off += D
                    # ------------- pool + final linear -------------
                    with tc.tile_pool(name="fin", bufs=1) as finp, \
                            tc.tile_pool(name="finps", bufs=1,
                                         space="PSUM") as fpp:
                        poolsb = finp.tile([HID, GPOOL], dt.float32)
                        nc.vector.tensor_copy(poolsb[:], poolps[:])
                        nc.sync.dma_start(cc_in[:, :], poolsb[:])
                        nc.gpsimd.collective_compute(
                            "AllReduce", Alu.add, replica_groups=rg,
                            ins=[cc_in[:, :]], outs=[cc_out[:, :]])
                        psum_t = finp.tile([HID, GPOOL], dt.float32)
                        nc.sync.dma_start(psum_t[:], cc_out[:, :])
                        mean_t = finp.tile([HID, GPOOL], dt.float32)
                        nc.vector.tensor_tensor(
                            mean_t[:], psum_t[:],
                            rc_t[:], Alu.mult)
                        psO = fpp.tile([GPOOL, OUT], dt.float32)
                        nc.tensor.matmul(psO[:], mean_t[:], wl_t[:],
                                         start=True, stop=True)
                        out_t = finp.tile([GPOOL, OUT], dt.float32)
                        if HASBL:
                            nc.vector.tensor_tensor(out_t[:], psO[:], bl_t[:],
                                                    Alu.add)
                        else:
                            nc.vector.tensor_copy(out_t[:], psO[:])
                        nc.sync.dma_start(t_out[:, :], out_t[:])
            if PHASES < 4:
                with tc.tile_pool(name='dummy', bufs=1) as dp:
                    d = dp.tile([GPOOL, OUT], dt.float32)
                    nc.vector.memset(d[:], 0.0)
                    nc.sync.dma_start(t_out[:, :], d[:])

    nc.compile()
    return nc


def core_inputs(prep, c):
    cd = prep["cores"][c]
    sc_D1, sc_D2 = prep["D1"], prep["D2"]
    NB1, NB2 = sum(sc_D1), sum(sc_D2)
    n1 = max(8 * NB1, 8)
    n2 = max(8 * NB2, 8)

    def padcols(a, cols, dtype):
        if a.shape[1] == cols:
            return np.ascontiguousarray(a)
        out = np.zeros((a.shape[0], cols), dtype)
        out[:, :a.shape[1]] = a
        return out

    return dict(
        xT=np.ascontiguousarray(cd["xT"]),
        w1=prep["W1"], w2ext=prep["W2ext"], wl=prep["Wl"],
        a1x=prep["a1x"],
        b1=prep["b1"], b2=prep["b2"], bl=prep["bl"], rcnt=prep["rcnt"],
        patch1=prep["patch1"], patch2=prep["patch2"], ident=prep["ident"],
        mpool=np.ascontiguousarray(cd["mpool"]),
        idx1=padcols(cd["w_idx1"], n1, np.int16),
        idx2=padcols(cd["w_idx2"], n2, np.int16),
        ue1=padcols(cd["ue1w"], max(NB1, 1), np.float32).astype(bf16),
        uo1=padcols(cd["uo1w"], max(NB1, 1), np.float32).astype(bf16),
        pme2=padcols(cd["pme2"], max(NB2, 1), np.float32),
        scat1=cd["w_scat1"],
    )


_CACHE = {}


def kernel(**inputs):
    from concourse.bass_utils import run_bass_kernel_spmd

    inputs = {k: np.asarray(v) for k, v in inputs.items()}
    prep = host_prep(**inputs)
    sc = make_sched(prep)
    sc["D1"] = prep["D1"]
    sc["D2"] = prep["D2"]
    key = str(sc)
    if key not in _CACHE:
        _CACHE[key] = build_bass(sc)
    nc = _CACHE[key]
    in_maps = [core_inputs(prep, c) for c in range(NCORES)]
    res = run_bass_kernel_spmd(nc, in_maps, list(range(NCORES)))
    return np.asarray(res.results[0]["out"], np.float32)


# revision 49
# speedup vs baseline: 1.2095x; 1.2095x over previous
# Self-contained 8-core Trainium2 Bass kernel for the 2-layer GAT + mean-pool
# problem (nn_GAT_83820581749190).
#
# Sharding: destination nodes (and all their incident edges) are partitioned
# across the 8 cores, so each layer's attention softmax and aggregation
# complete locally per core. Each core builds a replicated layer-1 feature
# table h1 (bf16, 256-byte logical rows) in HBM with a replicated x @ W1
# matmul, then edge-gathers PAIRS of rows (512B per descriptor, index =
# row//2, int16-safe) with the GPSIMD dma_gather custom op; host-precomputed
# parity masks select the correct half downstream. Attention logits are
# computed on-chip (DVE dot with a_src/a_dst), the edge softmax runs without
# segment-max (logits are small; pad slots use a patch row whose h gives
# al_src=-100), and aggregation is identity-matmul PSUM accumulation
# (destinations on partitions via degree-bucketed groups of 128).
# Layer-2 features are exchanged with an AllGather; the same paired-row
# gather runs against the fp32 layer-2 table; mean-pool is a matmul against
# a host-built one-hot graph matrix plus a tiny AllReduce.
import numpy as np
import ml_dtypes

N = 50000
E = 800000
IN = 128
HID = 32
HEADS = 4
OUT = 10
GPOOL = 64
NEG = 0.2
NCORES = 8
S = N // NCORES
SPECIAL1 = N          # layer-1 patch row (h chosen so h . a_src = -100)
SPECIAL2 = 0          # layer-2 patch row (al_src column = -100)
SPECIAL_ALS = -100.0
SB_BLOCK_BUDGET = 24  # max gather blocks per superblock
XCHUNK = 512
PHASES = 99
L1STEP = 99

bf16 = ml_dtypes.bfloat16


def _ceil_to(v, m):
    return (v + m - 1) // m * m


# ======================= host prep =========================================

def _build_layer(src, dstl):
    deg = np.bincount(dstl, minlength=S)
    P = np.argsort(-deg, kind="stable")
    Ppos = np.empty(S, np.int64)
    Ppos[P] = np.arange(S)
    ng = (S + 127) // 128
    D = np.zeros(ng, np.int64)
    dp = deg[P]
    for g in range(ng):
        D[g] = dp[g * 128:(g + 1) * 128].max()
    assert (D > 0).all()
    return dict(src=src, dstl=dstl, deg=deg, P=P, Ppos=Ppos, D=D)


def _emit_slots(l, Dg, row_of_src, special_row):
    """Per group g: rows[g] [D[g],128] of table ROW ids (special_row pads),
    plus slot2cmp mapping output slots -> compacted dst ids."""
    NG = len(Dg)
    Ppos = l["Ppos"]
    nreal = S
    slot2cmp = np.full(NG * 128, -1, np.int64)
    slot2cmp[:nreal] = np.arange(nreal)
    rows = [np.full((int(Dg[g]), 128), special_row, np.int64)
            for g in range(NG)]
    slot_of_edge = Ppos[l["dstl"]]
    order = np.argsort(slot_of_edge, kind="stable")
    so = slot_of_edge[order]
    sr = row_of_src[l["src"][order]]
    jj = np.arange(len(so)) - np.searchsorted(so, so, side="left")
    gg, kk = so // 128, so % 128
    for g in range(NG):
        sel = gg == g
        if sel.any():
            rows[g][jj[sel], kk[sel]] = sr[sel]
    return rows, slot2cmp


def _emit_slots_m(l, Dg, row_of, mate, special_row):
    """Slot emit with co-pair merging: consecutive same-dst edges whose
    sources are mates share one slot (both halves used)."""
    NG = len(Dg)
    Ppos = l["Ppos"]
    slot2cmp = np.full(NG * 128, -1, np.int64)
    slot2cmp[:S] = np.arange(S)
    rows = [np.full((int(Dg[g]), 128), special_row, np.int64)
            for g in range(NG)]
    ue = [np.ones((int(Dg[g]), 128), np.float32) for g in range(NG)]
    uo = [np.zeros((int(Dg[g]), 128), np.float32) for g in range(NG)]
    order = np.argsort(l["dstl"] * (N + 1) + l["src"], kind="stable")
    ds = l["dstl"][order]
    ss = l["src"][order]
    n = len(ds)
    fill = np.zeros(S, np.int64)
    i = 0
    while i < n:
        d = ds[i]
        u = ss[i]
        if i + 1 < n and ds[i + 1] == d and mate[u] == ss[i + 1]:
            r = row_of[u]
            assert r // 2 == row_of[ss[i + 1]] // 2
            rv, e_, o_ = (r // 2) * 2, 1.0, 1.0
            i += 2
        else:
            rv = row_of[u]
            e_, o_ = 1.0 - (rv % 2), float(rv % 2)
            i += 1
        slot = Ppos[d]
        g, k = slot // 128, slot % 128
        j = fill[d]
        fill[d] += 1
        rows[g][j, k] = rv
        ue[g][j, k] = e_
        uo[g][j, k] = o_
    assert (fill == l["deg"]).all()
    return rows, ue, uo, slot2cmp


def _wrap_masks(arrs):
    segs = [np.ascontiguousarray(a.T) for a in arrs if a.size]
    return (np.concatenate(segs, axis=1).astype(np.float32) if segs
            else np.zeros((128, 0), np.float32))


def _wrap16(idx):
    """[n] -> [128, n//16] int16: idx i at [i%16, i//16], replicated x8."""
    n = len(idx)
    assert n % 16 == 0
    w = np.ascontiguousarray(np.asarray(idx).reshape(n // 16, 16).T)
    w = w.astype(np.int16)
    return np.tile(w, (8, 1))


def _wrap_rows(rows_arrs):
    """idx stream (row//2) wrapped, plus even-parity masks [128, NB]."""
    idx_segs = []
    pme_segs = []
    for a in rows_arrs:
        if a.size:
            assert (a // 2 <= 32767).all()
            idx_segs.append(_wrap16((a // 2).reshape(-1)))
            pme_segs.append(np.ascontiguousarray((1 - (a % 2)).T))
    w_idx = (np.concatenate(idx_segs, axis=1) if idx_segs
             else np.zeros((128, 0), np.int16))
    pme = (np.concatenate(pme_segs, axis=1).astype(np.float32) if pme_segs
           else np.zeros((128, 0), np.float32))
    return w_idx, pme


def host_prep(x, edge_index, batch, W1, a1_src, a1_dst, b1, W2, a2_src, a2_dst,
              b2, Wl, bl):
    x = np.asarray(x, np.float32)
    edge_index = np.asarray(edge_index, np.int64)
    batch = np.asarray(batch, np.int64)
    src_all = np.concatenate([edge_index[0], np.arange(N, dtype=np.int64)])
    dst_all = np.concatenate([edge_index[1], np.arange(N, dtype=np.int64)])
    owner = dst_all // S

    a1_src = np.asarray(a1_src, np.float32)
    a1_dst = np.asarray(a1_dst, np.float32)
    W1 = np.asarray(W1, np.float32)
    W2 = np.asarray(W2, np.float32)
    W2ext = np.concatenate(
        [W2, W2 @ np.asarray(a2_src, np.float32)[0][:, None],
         W2 @ np.asarray(a2_dst, np.float32)[0][:, None]], axis=1)  # [128,34]

    # a1x: [0:128]=a_src flat, [128:256]=a_src flat, [256:384]=a_dst flat
    asf = a1_src.reshape(-1)
    adf = a1_dst.reshape(-1)
    a1x = np.tile(np.concatenate([asf, asf, adf])[None, :], (128, 1))

    # layer-1 patch row: h with h . a_src[h] = -100 for every head
    hp = np.concatenate([SPECIAL_ALS * a1_src[h] / (a1_src[h] ** 2).sum()
                         for h in range(HEADS)])
    assert np.abs(hp).max() < 1e4
    patch1 = np.tile(hp[None, :], (1, 1))

    cores = [dict(c=c) for c in range(NCORES)]
    for cd in cores:
        c = cd["c"]
        m = owner == c
        cd["src"] = src_all[m]
        cd["dstl"] = dst_all[m] - c * S

    # ---------- layer 1 ----------
    # Co-pair foreign sources that feed the same destination so both edges
    # share one 512B pair-descriptor (slot uses BOTH halves).
    for cd in cores:
        c = cd["c"]
        src, dstl = cd["src"], cd["dstl"]
        foreign = (src // S) != c
        order = np.argsort(dstl * (N + 1) + src, kind="stable")
        ds, ss, fs = dstl[order], src[order], foreign[order]
        mate = np.full(N, -1, np.int64)
        same = (ds[:-1] == ds[1:]) & fs[:-1] & fs[1:] & (ss[:-1] != ss[1:])
        cand_u = ss[:-1][same]
        cand_v = ss[1:][same]
        for u, v in zip(cand_u, cand_v):
            if mate[u] < 0 and mate[v] < 0:
                mate[u] = v
                mate[v] = u
        deg = np.bincount(dstl, minlength=S)
        merged = mate[cand_u] == cand_v
        dmerge = np.bincount(ds[:-1][same][merged], minlength=S)
        degp = deg - dmerge
        P = np.argsort(-degp, kind="stable")
        Ppos = np.empty(S, np.int64)
        Ppos[P] = np.arange(S)
        ng = (S + 127) // 128
        D = np.zeros(ng, np.int64)
        dp = degp[P]
        for g in range(ng):
            D[g] = dp[g * 128:(g + 1) * 128].max()
        cd["l1"] = dict(src=src, dstl=dstl, deg=degp, P=P, Ppos=Ppos, D=D)
        cd["mate1"] = mate
        pos_of = np.empty(N, np.int64)
        own_mask = np.zeros(N, bool)
        own_mask[c * S:(c + 1) * S] = True
        pos_of[c * S:(c + 1) * S] = Ppos
        fnodes = np.where(~own_mask)[0]
        is_m = mate[fnodes] >= 0
        mlist = fnodes[is_m]
        lo = mlist[mlist < mate[mlist]]
        nxt = S
        for u in lo:
            pos_of[u] = nxt
            pos_of[mate[u]] = nxt + 1
            nxt += 2
        singles = fnodes[~is_m]
        pos_of[singles] = nxt + np.arange(len(singles))
        cd["row_of"] = pos_of
    NG1 = max(len(cd["l1"]["D"]) for cd in cores)
    D1 = np.zeros(NG1, np.int64)
    for cd in cores:
        d = cd["l1"]["D"]
        D1[:len(d)] = np.maximum(D1[:len(d)], d)
    for cd in cores:
        cd["rows1"], cd["ue1"], cd["uo1"], cd["slot2cmp1"] = _emit_slots_m(
            cd["l1"], D1, cd["row_of"], cd["mate1"], SPECIAL1)

    # ---------- layer 2 ----------
    # layer-2 features live in a blocked bf16 table: core c's partition p,
    # group g at flat row (c*128+p)*NG2 + g (64 bf16 each; pairs of flat
    # rows share one 256B gather descriptor)
    for cd in cores:
        cd["l2"] = _build_layer(cd["src"], cd["dstl"])
    NG2 = max(len(cd["l2"]["D"]) for cd in cores)
    D2 = np.zeros(NG2, np.int64)
    for cd in cores:
        d = cd["l2"]["D"]
        D2[:len(d)] = np.maximum(D2[:len(d)], d)
    flat2_of = np.empty(N, np.int64)
    for cd in cores:
        c = cd["c"]
        q = cd["l2"]["Ppos"]
        flat2_of[c * S:(c + 1) * S] = \
            (c * 128 + q % 128) * NG2 + q // 128
    for cd in cores:
        c = cd["c"]
        special2 = (c * 128 + S % 128) * NG2 + S // 128  # own trash row
        cd["rows2"], cd["slot2cmp2"] = _emit_slots(
            cd["l2"], D2, flat2_of, special2)

    # ---------- aux ----------
    cnt = np.bincount(batch, minlength=GPOOL).astype(np.float32)
    recip_cnt = (1.0 / np.maximum(cnt, 1.0)).astype(np.float32)

    XT_COLS = _ceil_to(N + 2, XCHUNK)
    for cd in cores:
        c = cd["c"]
        gids = batch[c * S:(c + 1) * S]
        Mp = np.zeros((NG2 * 128, GPOOL), np.float32)
        s2c = cd["slot2cmp2"]
        real = s2c >= 0
        Mp[np.where(real)[0], gids[cd["l2"]["P"][s2c[real]]]] = 1.0
        cd["mpool"] = Mp.astype(bf16)

        s2c1 = cd["slot2cmp1"]
        tgt = np.full(len(s2c1), S, np.int64)  # trash row for dummy slots
        r1 = s2c1 >= 0
        tgt[r1] = cd["l2"]["Ppos"][cd["l1"]["P"][s2c1[r1]]]

        xt = np.zeros((IN, XT_COLS), np.float32)
        xt[:, cd["row_of"]] = x.T
        cd["xT"] = xt.astype(bf16)

        idx1_segs = [_wrap16((a // 2).reshape(-1)) for a in cd["rows1"]
                     if a.size]
        cd["w_idx1"] = (np.concatenate(idx1_segs, axis=1) if idx1_segs
                        else np.zeros((128, 0), np.int16))
        cd["ue1w"] = _wrap_masks(cd["ue1"])
        cd["uo1w"] = _wrap_masks(cd["uo1"])
        cd["w_idx2"], cd["pme2"] = _wrap_rows(cd["rows2"])
        cd["w_scat1"] = _wrap16(tgt)

    # written over the trash row after the scatter: al_src=-100 kills pads
    patch2 = np.zeros((1, 64), np.float32)
    patch2[0, 32] = SPECIAL_ALS

    return dict(cores=cores,
                D1=[int(v) for v in D1], D2=[int(v) for v in D2],
                W1=W1.astype(bf16), W2ext=W2ext.astype(bf16),
                Wl=np.asarray(Wl, np.float32),
                a1x=a1x.astype(bf16),
                b1=np.tile(np.asarray(b1, np.float32).reshape(1, -1),
                           (128, 1)),
                b2=np.tile(np.asarray(b2, np.float32).reshape(1, -1),
                           (128, 1)),
                bl=np.tile(np.asarray(bl, np.float32).reshape(1, -1),
                           (GPOOL, 1)),
                rcnt=np.tile(recip_cnt.reshape(1, -1), (HID, 1)),
                patch1=patch1.astype(bf16), patch2=patch2,
                ident=np.eye(128, dtype=bf16))


def _pack_superblocks(D, budget=SB_BLOCK_BUDGET):
    sbs, cur, tot = [], [], 0
    for g in range(len(D)):
        d = int(D[g])
        if cur and tot + d > budget:
            sbs.append(cur)
            cur, tot = [], 0
        cur.append(g)
        tot += d
    if cur:
        sbs.append(cur)
    return sbs


def make_sched(prep):
    D1, D2 = prep["D1"], prep["D2"]
    return dict(D1=D1, D2=D2,
                SB1=_pack_superblocks(D1), SB2=_pack_superblocks(D2),
                HASB1=bool(np.any(prep["b1"])), HASB2=bool(np.any(prep["b2"])),
                HASBL=bool(np.any(prep["bl"])))


# ======================= bass kernel =======================================

def build_bass(sc):
    import concourse.bacc as bacc
    import concourse.tile as tile
    import concourse.mybir as mybir
    from concourse.library_config import mlp

    dt = mybir.dt
    Alu = mybir.AluOpType
    Act = mybir.ActivationFunctionType
    Axis = mybir.AxisListType

    D1, D2 = sc["D1"], sc["D2"]
    SB1, SB2 = sc["SB1"], sc["SB2"]
    HASB1 = sc.get("HASB1", True)
    HASB2 = sc.get("HASB2", True)
    HASBL = sc.get("HASBL", True)
    NG1, NG2 = len(D1), len(D2)
    XT_COLS = _ceil_to(N + 2, XCHUNK)
    NCHUNK = XT_COLS // XCHUNK
    SH2_ROWS = _ceil_to(S + 2, 128)
    NB1 = sum(D1)
    NB2 = sum(D2)
    o1 = np.concatenate([[0], np.cumsum(D1)]).astype(int)
    o2 = np.concatenate([[0], np.cumsum(D2)]).astype(int)

    nc = bacc.Bacc("TRN2", target_bir_lowering=False, debug=False,
                   num_devices=NCORES, num_swdge_queues=4)

    t_xT = nc.dram_tensor("xT", [IN, XT_COLS], dt.bfloat16,
                          kind="ExternalInput")
    t_w1 = nc.dram_tensor("w1", [IN, IN], dt.bfloat16, kind="ExternalInput")
    t_w2 = nc.dram_tensor("w2ext", [IN, 34], dt.bfloat16,
                          kind="ExternalInput")
    t_wl = nc.dram_tensor("wl", [HID, OUT], dt.float32, kind="ExternalInput")
    t_a1x = nc.dram_tensor("a1x", [128, 384], dt.bfloat16,
                           kind="ExternalInput")
    t_b1 = nc.dram_tensor("b1", [128, HEADS * HID], dt.float32,
                          kind="ExternalInput")
    t_b2 = nc.dram_tensor("b2", [128, HID], dt.float32, kind="ExternalInput")
    t_bl = nc.dram_tensor("bl", [GPOOL, OUT], dt.float32,
                          kind="ExternalInput")
    t_rcnt = nc.dram_tensor("rcnt", [HID, GPOOL], dt.float32,
                            kind="ExternalInput")
    t_patch1 = nc.dram_tensor("patch1", [1, 128], dt.bfloat16,
                              kind="ExternalInput")
    t_patch2 = nc.dram_tensor("patch2", [1, 64], dt.float32,
                              kind="ExternalInput")
    t_ident = nc.dram_tensor("ident", [128, 128], dt.bfloat16,
                             kind="ExternalInput")
    t_mpool = nc.dram_tensor("mpool", [NG2 * 128, GPOOL], dt.bfloat16,
                             kind="ExternalInput")
    n1 = max(8 * NB1, 8)
    n2 = max(8 * NB2, 8)
    t_i1 = nc.dram_tensor("idx1", [128, n1], dt.int16, kind="ExternalInput")
    t_i2 = nc.dram_tensor("idx2", [128, n2], dt.int16, kind="ExternalInput")
    t_ue1 = nc.dram_tensor("ue1", [128, max(NB1, 1)], dt.bfloat16,
                           kind="ExternalInput")
    t_uo1 = nc.dram_tensor("uo1", [128, max(NB1, 1)], dt.bfloat16,
                           kind="ExternalInput")
    t_pm2 = nc.dram_tensor("pme2", [128, max(NB2, 1)], dt.float32,
                           kind="ExternalInput")
    t_scat1 = nc.dram_tensor("scat1", [128, 8 * NG1], dt.int16,
                             kind="ExternalInput")
    t_out = nc.dram_tensor("out", [GPOOL, OUT], dt.float32,
                           kind="ExternalOutput")

    rg = [list(range(NCORES))]
    _qc = [0]

    def nextq():
        _qc[0] = (_qc[0] + 1) % 4
        return _qc[0]

    with tile.TileContext(nc) as tc:
        with (
            tc.tile_pool(name="const", bufs=1) as constp,
            tc.tile_pool(name="pre", bufs=1) as prep_pool,
            tc.tile_pool(name="dram", bufs=1, space="DRAM") as dramp,
        ):
            nc.gpsimd.load_library(mlp)

            # logical row-major tables; gathers view them as paired rows
            table1 = dramp.tile([XT_COLS, 128], dt.bfloat16, tag="table1")
            h2b = dramp.tile([128, NG2 * 64], dt.bfloat16, tag="h2b")
            table2b = dramp.tile([128 * NCORES * NG2 * 64], dt.bfloat16,
                                 tag="table2b")
            h2sh = dramp.tile([SH2_ROWS, 64], dt.float32, tag="h2sh")
            cc_in = dramp.tile([HID, GPOOL], dt.float32, tag="ccin")
            cc_out = dramp.tile([HID, GPOOL], dt.float32, tag="ccout")

            w1_t = constp.tile([IN, IN], dt.bfloat16)
            nc.sync.dma_start(w1_t[:], t_w1[:])
            w2_t = constp.tile([IN, 34], dt.bfloat16)
            nc.sync.dma_start(w2_t[:], t_w2[:])
            wl_t = constp.tile([HID, OUT], dt.float32)
            nc.sync.dma_start(wl_t[:], t_wl[:])
            a1x_t = constp.tile([128, 384], dt.bfloat16)
            nc.sync.dma_start(a1x_t[:], t_a1x[:])
            b1_t = constp.tile([128, HEADS * HID], dt.float32)
            nc.sync.dma_start(b1_t[:], t_b1[:])
            b2_t = constp.tile([128, HID], dt.float32)
            nc.sync.dma_start(b2_t[:], t_b2[:])
            bl_t = constp.tile([GPOOL, OUT], dt.float32)
            nc.sync.dma_start(bl_t[:], t_bl[:])
            rc_t = constp.tile([HID, GPOOL], dt.float32)
            nc.sync.dma_start(rc_t[:], t_rcnt[:])
            id_t = constp.tile([128, 128], dt.bfloat16)
            nc.sync.dma_start(id_t[:], t_ident[:])

            # preload all gather indices and parity masks
            i1_all = prep_pool.tile([128, n1], dt.int16)
            nc.sync.dma_start(i1_all[:], t_i1[:])
            ue1_t = prep_pool.tile([128, max(NB1, 1)], dt.bfloat16)
            nc.sync.dma_start(ue1_t[:], t_ue1[:])
            uo1_t = prep_pool.tile([128, max(NB1, 1)], dt.bfloat16)
            nc.sync.dma_start(uo1_t[:], t_uo1[:])
            i2_all = prep_pool.tile([128, n2], dt.int16)
            pm2_t = prep_pool.tile([128, max(NB2, 1)], dt.float32)
            scat1_t = prep_pool.tile([128, 8 * NG1], dt.int16)
            nc.sync.dma_start(scat1_t[:], t_scat1[:])
            ald1_t = prep_pool.tile([128, NG1, 4], dt.float32)
            ald2_t = prep_pool.tile([128, NG2, 1], dt.float32)

            # zero the scatter_add target
            with tc.tile_pool(name="zp", bufs=1) as zp:
                z_t = zp.tile([128, SH2_ROWS // 128 * 64], dt.float32)
                nc.vector.memset(z_t[:], 0.0)
                nc.sync.dma_start(
                    h2sh[:, :].rearrange("(p k) e -> p (k e)", p=128), z_t[:])

            # ---------------- phase X: build table1 ----------------
            with (
                tc.tile_pool(name="xload", bufs=3) as xlp,
                tc.tile_pool(name="xout", bufs=3) as xop,
                tc.tile_pool(name="xpsum", bufs=4, space="PSUM") as xpp,
            ):
                for t in range(NCHUNK):
                    # alternate loads/writes across the two HWDGE rings so
                    # neither ring serializes the whole 25.6MB stream
                    ld_eng = nc.sync if t % 2 == 0 else nc.scalar
                    st_eng = nc.scalar if t % 2 == 0 else nc.sync
                    xt_t = xlp.tile([IN, XCHUNK], dt.bfloat16, tag="xt")
                    ld_eng.dma_start(xt_t[:],
                                     t_xT[:, t * XCHUNK:(t + 1) * XCHUNK])
                    o_t = xop.tile([128, 4, 128], dt.bfloat16, tag="xo")
                    for k in range(4):
                        ps = xpp.tile([128, 128], dt.float32, tag="xp")
                        nc.tensor.matmul(ps[:], xt_t[:, k * 128:(k + 1) * 128],
                                         w1_t[:], start=True, stop=True)
                        if k % 2 == 0:
                            nc.vector.tensor_copy(o_t[:, k, :], ps[:])
                        else:
                            nc.scalar.activation(o_t[:, k, :], ps[:],
                                                 Act.Copy)
                        gix = 4 * t + k
                        if gix < NG1:
                            # own-row ald = h . a_dst, straight off the PSUM
                            ap_t = xop.tile([128, 128], dt.bfloat16,
                                            tag="apr")
                            nc.vector.tensor_tensor(
                                ap_t[:], ps[:], a1x_t[:, 256:384], Alu.mult)
                            nc.vector.tensor_reduce(
                                ald1_t[:, gix, :],
                                ap_t[:].rearrange("p (h c) -> p h c", h=4),
                                axis=Axis.X, op=Alu.add)
                    st_eng.dma_start(
                        table1[t * XCHUNK:(t + 1) * XCHUNK, :].rearrange(
                            "(k p) e -> p k e", p=128), o_t[:])
            with tc.tile_pool(name="patchp", bufs=1) as pp:
                p1_t = pp.tile([1, 128], dt.bfloat16)
                nc.sync.dma_start(p1_t[:], t_patch1[:])
                nc.sync.dma_start(table1[SPECIAL1:SPECIAL1 + 1, :],
                                  p1_t[0:1, :])
            p2_t = prep_pool.tile([1, 64], dt.float32)
            nc.sync.dma_start(p2_t[:], t_patch2[:])

            if PHASES >= 2:
                # ---------------- phase L1: edges ----------------
                tab1p = table1[:, :].rearrange("(a h) c -> a (h c)", h=2)
                Dmax1 = max(D1)
                NBSB1 = max(sum(D1[g] for g in sb) for sb in SB1)
                with (
                    tc.tile_pool(name="gath1", bufs=5) as gathp,
                    tc.tile_pool(name="als1", bufs=2) as alsp,
                    tc.tile_pool(name="small1", bufs=3) as smallp,
                    tc.tile_pool(name="epi1", bufs=3) as epip,
                    tc.tile_pool(name="scatp", bufs=1) as scatp,
                    tc.tile_pool(name="agg1", bufs=2, space="PSUM") as aggp,
                    tc.tile_pool(name="psT1", bufs=2, space="PSUM") as psTp,
                    tc.tile_pool(name="ps21", bufs=2, space="PSUM") as ps2p,
                ):
                    scat_t = scatp.tile([128, NG1, 64], dt.float32, tag="sc")
                    nc.vector.memset(scat_t[:], 0.0)
                    elu_all = scatp.tile([128, NG1, 128], dt.bfloat16,
                                         tag="eluall")
                    scat_done = [0]

                    def flush_scatter(upto):
                        g0s = scat_done[0]
                        ngk = upto - g0s
                        if ngk <= 0:
                            return
                        nc.gpsimd.dma_scatter_add(
                            h2sh[0:S + 1, :], scat_t[:, g0s:upto, :],
                            scat1_t[:, 8 * g0s:8 * upto],
                            128 * ngk, 128 * ngk, 64,
                            single_packet=False, queue_num=nextq())
                        scat_done[0] = upto

                    for sb in SB1:
                        g0 = sb[0]
                        nb = sum(D1[g] for g in sb)
                        boff = o1[g0]
                        gb_t = gathp.tile([128, NBSB1, 256],
                                          dt.bfloat16, tag="gb")
                        nc.gpsimd.dma_gather(
                            gb_t[:, :nb, :], tab1p,
                            i1_all[:, 8 * boff:8 * (boff + nb)],
                            128 * nb, 128 * nb, 256,
                            single_packet=False, queue_num=nextq())
                        if L1STEP < 2:
                            continue
                        # al_src for both pair-halves: prod + reduce
                        prod_t = alsp.tile([128, NBSB1, 256], dt.bfloat16,
                                           tag="prod")
                        als8_t = alsp.tile([128, NBSB1, 8], dt.float32,
                                           tag="als8")
                        nc.vector.tensor_tensor(
                            prod_t[:, :nb, :], gb_t[:, :nb, :],
                            a1x_t[:, 0:256].unsqueeze(1).broadcast_to(
                                (128, nb, 256)), Alu.mult)
                        nc.vector.tensor_reduce(
                            als8_t[:, :nb, :],
                            prod_t[:, :nb, :].rearrange(
                                "p b (j c) -> p b j c", j=8),
                            axis=Axis.X, op=Alu.add)
                        off = 0
                        for gi, g in enumerate(sb):
                            D = D1[g]
                            if L1STEP < 3:
                                off += D
                                continue
                            logit_t = smallp.tile([128, Dmax1, 8], dt.float32,
                                                  tag="lg")
                            exb_t = smallp.tile([128, Dmax1, 8], dt.bfloat16,
                                                tag="exb")
                            exe_t = smallp.tile([128, Dmax1, 4], dt.bfloat16,
                                                tag="exe")
                            exo_t = smallp.tile([128, Dmax1, 4], dt.bfloat16,
                                                tag="exo")
                            sum_t = smallp.tile([128, Dmax1, 4], dt.bfloat16,
                                                tag="sm")
                            den_t = smallp.tile([128, 4], dt.float32,
                                                tag="dn")
                            rec_t = smallp.tile([128, 4], dt.float32,
                                                tag="rc")
                            ald_ap = ald1_t[:, g, :]
                            ald_b = ald_ap.unsqueeze(1).broadcast_to(
                                (128, D, 4))
                            # independent logits for both pair halves
                            nc.vector.scalar_tensor_tensor(
                                logit_t[:, :D, 0:4],
                                als8_t[:, off:off + D, 0:4], 0.0,
                                ald_b, Alu.add, Alu.add)
                            nc.vector.scalar_tensor_tensor(
                                logit_t[:, :D, 4:8],
                                als8_t[:, off:off + D, 4:8], 0.0,
                                ald_b, Alu.add, Alu.add)
                            nc.vector.scalar_tensor_tensor(
                                logit_t[:, :D, :], logit_t[:, :D, :], NEG,
                                logit_t[:, :D, :], Alu.mult, Alu.max)
                            nc.scalar.activation(exb_t[:, :D, :],
                                                 logit_t[:, :D, :], Act.Exp)
                            ue_b = ue1_t[:, boff + off:boff + off + D
                                         ].unsqueeze(2).broadcast_to(
                                             (128, D, 4))
                            uo_b = uo1_t[:, boff + off:boff + off + D
                                         ].unsqueeze(2).broadcast_to(
                                             (128, D, 4))
                            nc.vector.tensor_tensor(
                                exe_t[:, :D, :], exb_t[:, :D, 0:4], ue_b,
                                Alu.mult)
                            nc.vector.tensor_tensor(
                                exo_t[:, :D, :], exb_t[:, :D, 4:8], uo_b,
                                Alu.mult)
                            nc.vector.tensor_tensor(
                                sum_t[:, :D, :], exe_t[:, :D, :],
                                exo_t[:, :D, :], Alu.add)
                            nc.vector.tensor_reduce(
                                den_t[:], sum_t[:, :D, :].transpose([0, 2, 1]),
                                axis=Axis.X, op=Alu.add)
                            nc.vector.reciprocal(rec_t[:], den_t[:])
                            if L1STEP < 4:
                                off += D
                                continue
                            h_e = gb_t[:, off:off + D, 0:128].rearrange(
                                "p b (h c) -> p b h c", h=4)
                            nc.vector.tensor_tensor(
                                h_e, h_e,
                                exe_t[:, :D, :].unsqueeze(3).broadcast_to(
                                    (128, D, 4, HID)), Alu.mult)
                            h_o = gb_t[:, off:off + D, 128:256].rearrange(
                                "p b (h c) -> p b h c", h=4)
                            nc.vector.tensor_tensor(
                                h_o, h_o,
                                exo_t[:, :D, :].unsqueeze(3).broadcast_to(
                                    (128, D, 4, HID)), Alu.mult)
                            if L1STEP < 5:
                                off += D
                                continue
                            agg = aggp.tile([128, 128], dt.float32, tag="agg")
                            for bi in range(2 * D):
                                rhs = gb_t[:, off + bi // 2,
                                           (bi % 2) * 128:(bi % 2 + 1) * 128]
                                nc.tensor.matmul(agg[:], id_t[:], rhs,
                                                 start=(bi == 0),
                                                 stop=(bi == 2 * D - 1))
                            scaled_t = epip.tile([128, 128], dt.float32,
                                                 tag="sd")
                            nc.vector.tensor_tensor(
                                scaled_t[:].rearrange("p (h c) -> p h c", h=4),
                                agg[:].rearrange("p (h c) -> p h c", h=4),
                                rec_t[:].unsqueeze(2).broadcast_to(
                                    (128, 4, HID)), Alu.mult)
                            if HASB1:
                                nc.vector.tensor_tensor(
                                    scaled_t[:], scaled_t[:], b1_t[:],
                                    Alu.add)
                            tmp_t = epip.tile([128, 128], dt.float32,
                                              tag="tm")
                            nc.scalar.activation(tmp_t[:], scaled_t[:],
                                                 Act.Relu, scale=-1.0)
                            nc.scalar.activation(tmp_t[:], tmp_t[:], Act.Exp,
                                                 scale=-1.0)
                            nc.vector.scalar_tensor_tensor(
                                elu_all[:, g, :], tmp_t[:], -1.0, scaled_t[:],
                                Alu.add, Alu.max)
                            off += D
                        # ---- pass 2 for this superblock's groups
                        if L1STEP >= 5:
                            for g in sb:
                                psT = psTp.tile([128, 128], dt.bfloat16,
                                                tag="pt")
                                nc.tensor.transpose(psT[:], elu_all[:, g, :],
                                                    id_t[:])
                                eluT_t = epip.tile([128, 128], dt.bfloat16,
                                                   tag="et")
                                nc.scalar.activation(eluT_t[:], psT[:],
                                                     Act.Copy)
                                ps2 = ps2p.tile([128, 34], dt.float32,
                                                tag="p2")
                                nc.tensor.matmul(ps2[:], eluT_t[:], w2_t[:],
                                                 start=True, stop=True)
                                if g % 2 == 0:
                                    nc.scalar.activation(scat_t[:, g, 0:34],
                                                         ps2[:], Act.Copy)
                                else:
                                    nc.vector.tensor_copy(scat_t[:, g, 0:34],
                                                          ps2[:])
                            # overlap the h2 scatter with the remaining
                            # superblocks (~12-group chunks)
                            if sb[-1] + 1 - scat_done[0] >= 12:
                                flush_scatter(sb[-1] + 1)
                    if L1STEP >= 6:
                        flush_scatter(NG1)
                        # pad slots scatter garbage into the trash row;
                        # overwrite with the al_src=-100 pad row before the
                        # L2 extraction reads it
                        nc.sync.dma_start(h2sh[S:S + 1, :], p2_t[0:1, :])
            if PHASES >= 3:
                # ---------------- exchange ----------------
                # pack own rows to blocked bf16 (also yields the ald column),
                # then AllGather the compact table
                with tc.tile_pool(name="aldtmp2", bufs=1) as atp:
                    atmp = atp.tile([128, NG2, 64], dt.float32)
                    nc.sync.dma_start(
                        atmp[:],
                        h2sh[0:128 * NG2, :].rearrange("(b p) e -> p b e",
                                                       p=128))
                    nc.vector.tensor_copy(ald2_t[:], atmp[:, :, 33:34])
                    atmpb = atp.tile([128, NG2, 64], dt.bfloat16)
                    nc.scalar.activation(atmpb[:], atmp[:], Act.Copy)
                    nc.sync.dma_start(
                        h2b[:, :], atmpb[:].rearrange("p b e -> p (b e)"))
                nc.gpsimd.collective_compute(
                    "AllGather", mybir.AluOpType.bypass, replica_groups=rg,
                    ins=[h2b[:, :]],
                    outs=[table2b[:].rearrange("(r x) -> r x",
                                               r=128 * NCORES)])

            if PHASES >= 4:
                # ---------------- phase L2: edges + pool ----------------
                # preloads below overlap the AllGather
                nc.sync.dma_start(i2_all[:], t_i2[:])
                nc.sync.dma_start(pm2_t[:], t_pm2[:])
                tab2p = table2b[:].rearrange("(y c) -> y c", c=128)
                Dmax2 = max(D2)
                NBSB2 = max(sum(D2[g] for g in sb) for sb in SB2)
                with (
                    tc.tile_pool(name="gath2", bufs=6) as gathp,
                    tc.tile_pool(name="small2", bufs=3) as smallp,
                    tc.tile_pool(name="epi2", bufs=3) as epip,
                    tc.tile_pool(name="agg2", bufs=2, space="PSUM") as aggp,
                    tc.tile_pool(name="poolps", bufs=1,
                                 space="PSUM") as poolpp,
                    tc.tile_pool(name="mp2", bufs=3) as mpp,
                ):
                    poolps = poolpp.tile([HID, GPOOL], dt.float32)
                    h2p_all = mpp.tile([128, NG2, HID], dt.bfloat16,
                                       tag="h2pall", bufs=1)
                    for sb in SB2:
                        g0 = sb[0]
                        nb = sum(D2[g] for g in sb)
                        boff = o2[g0]
                        gb_t = gathp.tile([128, NBSB2, 128], dt.bfloat16,
                                          tag="gb")
                        nc.gpsimd.dma_gather(
                            gb_t[:, :nb, :], tab2p,
                            i2_all[:, 8 * boff:8 * (boff + nb)],
                            128 * nb, 128 * nb, 128, single_packet=False,
                            queue_num=nextq())
                        off = 0
                        for gi, g in enumerate(sb):
                            D = D2[g]
                            logit_t = smallp.tile([128, Dmax2, 1], dt.float32,
                                                  tag="lg")
                            t2_t = smallp.tile([128, Dmax2, 1], dt.float32,
                                               tag="t2")
                            exf_t = smallp.tile([128, Dmax2, 1], dt.float32,
                                                tag="exf")
                            exe_t = smallp.tile([128, Dmax2, 1], dt.float32,
                                                tag="exe")
                            exo_t = smallp.tile([128, Dmax2, 1], dt.float32,
                                                tag="exo")
                            den_t = smallp.tile([128, 1], dt.float32,
                                                tag="dn")
                            rec_t = smallp.tile([128, 1], dt.float32,
                                                tag="rc")
                            ald_ap = ald2_t[:, g, :]
                            pmg = pm2_t[:, boff + off:boff + off + D]
                            pmg_b = pmg.unsqueeze(2)
                            # als = odd - (odd-even)*pme
                            nc.vector.tensor_tensor(
                                t2_t[:, :D, :],
                                gb_t[:, off:off + D, 96:97],
                                gb_t[:, off:off + D, 32:33], Alu.subtract)
                            nc.vector.tensor_tensor(
                                t2_t[:, :D, :], t2_t[:, :D, :], pmg_b,
                                Alu.mult)
                            nc.vector.tensor_tensor(
                                logit_t[:, :D, :],
                                gb_t[:, off:off + D, 96:97],
                                t2_t[:, :D, :], Alu.subtract)
                            nc.vector.tensor_scalar(
                                logit_t[:, :D, :], logit_t[:, :D, :],
                                ald_ap, None, Alu.add)
                            nc.vector.scalar_tensor_tensor(
                                logit_t[:, :D, :], logit_t[:, :D, :], NEG,
                                logit_t[:, :D, :], Alu.mult, Alu.max)
                            nc.scalar.activation(exf_t[:, :D, :],
                                                 logit_t[:, :D, :], Act.Exp)
                            nc.vector.tensor_reduce(
                                den_t[:], exf_t[:, :D, :].transpose([0, 2, 1]),
                                axis=Axis.X, op=Alu.add)
                            nc.vector.reciprocal(rec_t[:], den_t[:])
                            nc.vector.tensor_tensor(
                                exe_t[:, :D, :], exf_t[:, :D, :], pmg_b,
                                Alu.mult)
                            nc.vector.tensor_tensor(
                                exo_t[:, :D, :], exf_t[:, :D, :],
                                exe_t[:, :D, :], Alu.subtract)
                            exh_t = smallp.tile([128, Dmax2, 2, HID],
                                                dt.bfloat16, tag="exh")
                            nc.vector.tensor_tensor(
                                exh_t[:, :D, 0, :],
                                gb_t[:, off:off + D, 0:HID],
                                exe_t[:, :D, :].broadcast_to(
                                    (128, D, HID)), Alu.mult)
                            nc.vector.tensor_tensor(
                                exh_t[:, :D, 1, :],
                                gb_t[:, off:off + D, 64:64 + HID],
                                exo_t[:, :D, :].broadcast_to(
                                    (128, D, HID)), Alu.mult)
                            agg = aggp.tile([128, HID], dt.float32, tag="agg")
                            for bi in range(2 * D):
                                nc.tensor.matmul(
                                    agg[:], id_t[:],
                                    exh_t[:, bi // 2, bi % 2, :],
                                    start=(bi == 0),
                                    stop=(bi == 2 * D - 1))
                            scaled_t = epip.tile([128, HID], dt.float32,
                                                 tag="sd")
                            nc.vector.tensor_scalar(scaled_t[:], agg[:],
                                                    rec_t[:], None, Alu.mult)
                            if HASB2:
                                nc.vector.tensor_tensor(
                                    scaled_t[:], scaled_t[:], b2_t[:],
                                    Alu.add)
                            tmp_t = epip.tile([128, HID], dt.float32,
                                              tag="tm")
                            nc.scalar.activation(tmp_t[:], scaled_t[:],
                                                 Act.Relu, scale=-1.0)
                            nc.scalar.activation(tmp_t[:], tmp_t[:], Act.Exp,
                                                 scale=-1.0)
                            nc.vector.scalar_tensor_tensor(
                                h2p_all[:, g, :], tmp_t[:], -1.0, scaled_t[:],
                                Alu.add, Alu.max)
                            mp_t = mpp.tile([128, GPOOL], dt.bfloat16,
                                            tag="mp")
                            nc.sync.dma_start(
                                mp_t[:], t_mpool[g * 128:(g + 1) * 128, :])
                            nc.tensor.matmul(poolps[:], h2p_all[:, g, :],
                                             mp_t[:], start=(g == 0),
                                             stop=(g == NG2 - 1))
                            off += D
                    # ------------- pool + final linear -------------
                    with tc.tile_pool(name="fin", bufs=1) as finp, \
                            tc.tile_pool(name="finps", bufs=1,
                                         space="PSUM") as fpp:
                        poolsb = finp.tile([HID, GPOOL], dt.float32)
                        nc.vector.tensor_copy(poolsb[:], poolps[:])
                        nc.sync.dma_start(cc_in[:, :], poolsb[:])
                        nc.gpsimd.collective_compute(
                            "AllReduce", Alu.add, replica_groups=rg,
                            ins=[cc_in[:, :]], outs=[cc_out[:, :]])
                        psum_t = finp.tile([HID, GPOOL], dt.float32)
                        nc.sync.dma_start(psum_t[:], cc_out[:, :])
                        mean_t = finp.tile([HID, GPOOL], dt.float32)
                        nc.vector.tensor_tensor(
                            mean_t[:], psum_t[:],
                            rc_t[:], Alu.mult)
                        psO = fpp.tile([GPOOL, OUT], dt.float32)
                        nc.tensor.matmul(psO[:], mean_t[:], wl_t[:],
                                         start=True, stop=True)
                        out_t = finp.tile([GPOOL, OUT], dt.float32)
                        if HASBL:
                            nc.vector.tensor_tensor(out_t[:], psO[:], bl_t[:],
                                                    Alu.add)
                        else:
                            nc.vector.tensor_copy(out_t[:], psO[:])
                        nc.sync.dma_start(t_out[:, :], out_t[:])
            if PHASES < 4:
                with tc.tile_pool(name='dummy', bufs=1) as dp:
                    d = dp.tile([GPOOL, OUT], dt.float32)
                    nc.vector.memset(d[:], 0.0)
                    nc.sync.dma_start(t_out[:, :], d[:])

    nc.compile()
    return nc


def core_inputs(prep, c):
    cd = prep["cores"][c]
    sc_D1, sc_D2 = prep["D1"], prep["D2"]
    NB1, NB2 = sum(sc_D1), sum(sc_D2)
    n1 = max(8 * NB1, 8)
    n2 = max(8 * NB2, 8)

    def padcols(a, cols, dtype):
        if a.shape[1] == cols:
            return np.ascontiguousarray(a)
        out = np.zeros((a.shape[0], cols), dtype)
        out[:, :a.shape[1]] = a
        return out

    return dict(
        xT=np.ascontiguousarray(cd["xT"]),
        w1=prep["W1"], w2ext=prep["W2ext"], wl=prep["Wl"],
        a1x=prep["a1x"],
        b1=prep["b1"], b2=prep["b2"], bl=prep["bl"], rcnt=prep["rcnt"],
        patch1=prep["patch1"], patch2=prep["patch2"], ident=prep["ident"],
        mpool=np.ascontiguousarray(cd["mpool"]),
        idx1=padcols(cd["w_idx1"], n1, np.int16),
        idx2=padcols(cd["w_idx2"], n2, np.int16),
        ue1=padcols(cd["ue1w"], max(NB1, 1), np.float32).astype(bf16),
        uo1=padcols(cd["uo1w"], max(NB1, 1), np.float32).astype(bf16),
        pme2=padcols(cd["pme2"], max(NB2, 1), np.float32),
        scat1=cd["w_scat1"],
    )


_CACHE = {}


def kernel(**inputs):
    from concourse.bass_utils import run_bass_kernel_spmd

    inputs = {k: np.asarray(v) for k, v in inputs.items()}
    prep = host_prep(**inputs)
    sc = make_sched(prep)
    sc["D1"] = prep["D1"]
    sc["D2"] = prep["D2"]
    key = str(sc)
    if key not in _CACHE:
        _CACHE[key] = build_bass(sc)
    nc = _CACHE[key]
    in_maps = [core_inputs(prep, c) for c in range(NCORES)]
    res = run_bass_kernel_spmd(nc, in_maps, list(range(NCORES)))
    return np.asarray(res.results[0]["out"], np.float32)
